# revision 13
# baseline (speedup 1.0000x reference)
"""FPN encoder (MTGNN/MAGNN-style) Trainium2 kernel.

Strategy:
 - Host: graph constructor (tiny, input-only-dependent, exact-tie-sensitive
   top-k) computed bit-exactly with jax-CPU in a subprocess; adjacencies are
   then row-normalized, transposed, padded to 1024 and replicated to all
   cores like weights. The linear FPN conv pyramid and the mixprop 1x1-conv
   channel mixes are folded (host-side weight preprocessing) into per-scale
   band matrices G0/G1/G2 of one composite temporal conv each, with biases as
   an extra constant-one row of x.
 - Device (8 cores, zero collectives): core (b, h) handles batch b and the
   h-th half of the time axis of every scale. Everything is node-major
   matmuls on the tensor engine in float32r:
       S2 = xa^T G2 ; u = A (S2) + xa^T G1 ; out_d = A u + xa^T G0
   with A-chunks as stationary operands, two hops per scale per direction.
 - Host gather: slice away node padding, fold the two directions' partial
   sums, reassemble the 5-tuple of full outputs.
"""

import os
import sys
import subprocess
import tempfile

import numpy as np

for _p in ('/opt/trn_rl_repo', '/root/.axon_site/_ro/trn_rl_repo'):
    if os.path.isdir(_p) and _p not in sys.path:
        sys.path.append(_p)

B, T, NNODE, C, D = 4, 120, 1000, 32, 40
KERNEL_SET = [14, 7, 6, 3]
PROPALPHA, ALPHA, TOPK = 0.05, 3.0, 20
SCALE_SET = [1.0, 0.8, 0.6, 0.5]
NPAD = 1024
L_LIST = [107, 101, 96, 94]
LH = [54, 51, 48, 47]          # per-scale half lengths (ceil(L/2))
F_LIST = [32 * l for l in LH]  # free width per scale: 1728,1632,1536,1504
FCH = [f // 4 for f in F_LIST]  # free chunk: 432,408,384,376 (256..512)
GLEN = [14, 20, 25, 27]        # composite conv kernel lengths

_CHILD = r'''
import sys, numpy as np
import jax, jax.numpy as jnp
pin, pout = sys.argv[1], sys.argv[2]
z = np.load(pin)
emb1, emb2 = jnp.asarray(z["emb1"]), jnp.asarray(z["emb2"])
l1w, l1b = jnp.asarray(z["lin1_w"]), jnp.asarray(z["lin1_b"])
l2w, l2b = jnp.asarray(z["lin2_w"]), jnp.asarray(z["lin2_b"])
NN, KK, AL = 1000, 20, 3.0
SC = [1.0, 0.8, 0.6, 0.5]
n1, n2 = emb1, emb2
rows = jnp.arange(NN)[:, None]
adjs = []
for i in range(4):
    n1 = jnp.tanh(AL * ((n1 * SC[i]) @ l1w[i].T + l1b[i]))
    n2 = jnp.tanh(AL * ((n2 * SC[i]) @ l2w[i].T + l2b[i]))
    a = n1 @ n2.T - n2 @ n1.T
    adj0 = jax.nn.relu(jnp.tanh(AL * a))
    _, t1 = jax.lax.top_k(adj0, KK)
    mask = jnp.zeros_like(adj0).at[rows, t1].set(1.0)
    adjs.append(np.asarray(adj0 * mask))
np.savez(pout, **{f"a{i}": adjs[i] for i in range(4)})
'''


def _graph_adjs(inputs):
    """Replicate reference.graph_construct bit-exactly on jax-CPU."""
    with tempfile.TemporaryDirectory() as td:
        pin = os.path.join(td, "in.npz")
        pout = os.path.join(td, "out.npz")
        np.savez(pin, emb1=inputs["emb1"], emb2=inputs["emb2"],
                 lin1_w=inputs["lin1_w"], lin1_b=inputs["lin1_b"],
                 lin2_w=inputs["lin2_w"], lin2_b=inputs["lin2_b"])
        env = dict(os.environ)
        env["JAX_PLATFORMS"] = "cpu"
        env.pop("TRN_TERMINAL_POOL_IPS", None)
        parts = []
        for chunk in (env.get("PYTHONPATH", ""), env.get("NIX_PYTHONPATH", "")):
            parts.extend(p for p in chunk.split(os.pathsep) if p)
        parts.extend(p for p in sys.path if p)
        env["PYTHONPATH"] = os.pathsep.join(dict.fromkeys(parts))
        r = subprocess.run([sys.executable, "-c", _CHILD, pin, pout],
                           env=env, capture_output=True, text=True, timeout=900)
        if r.returncode != 0:
            raise RuntimeError(f"graph subprocess failed:\n{r.stdout}\n{r.stderr}")
        z = np.load(pout)
        return [z[f"a{i}"] for i in range(4)]


def _composite_convs(inputs):
    """Compose the linear FPN pyramid into one temporal kernel per scale.

    Returns g_list[i] [32, GLEN[i]] and beta_list[i] [32] (fp64) such that
    scales[i][c, n, l] = sum_k g[c, k] x[l + k, n] + beta[c].
    """
    g_list, beta_list = [], []
    cur_g = None
    cur_beta = None
    for idx, k in enumerate(KERNEL_SET):
        w = np.asarray(inputs[f"msc_w{idx}"], dtype=np.float64)
        b = np.asarray(inputs[f"msc_b{idx}"], dtype=np.float64)
        if idx == 0:
            cur_g = w[:, 0, 0, :].copy()
            cur_beta = b.copy()
        else:
            kp = cur_g.shape[1]
            ng = np.zeros((C, kp + k - 1))
            for t2 in range(k):
                ng[:, t2:t2 + kp] += w[:, :, 0, t2] @ cur_g
            cur_beta = b + w[:, :, 0, :].sum(-1) @ cur_beta
            cur_g = ng
        g_list.append(cur_g.copy())
        beta_list.append(cur_beta.copy())
    return g_list, beta_list


def _band_matrix(gE, bE, scale, half):
    """[121, 32*LH] band matrix for one (scale, set, half): columns are
    (l_local, c); rows are input time (120) plus the constant-one bias row."""
    Ki = gE.shape[1]
    Lh = LH[scale]
    l0 = 0 if half == 0 else L_LIST[scale] - Lh
    Gm = np.zeros((T + 1, 32 * Lh), dtype=np.float64)
    gT = gE.T  # [Ki, 32]
    for ll in range(Lh):
        l = l0 + ll
        Gm[l:l + Ki, ll * 32:(ll + 1) * 32] = gT
    Gm[T, :] = np.tile(bE, Lh)
    return Gm.astype(np.float32)


def _host_prep(inputs):
    """All host-side preprocessing -> per-core input maps (minus xa)."""
    adjs = _graph_adjs(inputs)
    eye = np.eye(NNODE, dtype=np.float32)
    adjt = np.zeros((8, NPAD, NPAD), dtype=np.float32)
    # scale-0 compaction: the saturated-plateau top-k picks low column
    # indices, so the off-diagonal part of A_0 spans few node chunks.
    used = (adjs[0] > 0).any(0)
    ncol0 = int(np.nonzero(used)[0].max()) + 1 if used.any() else 1
    w0chunks = (ncol0 + 127) // 128
    if w0chunks >= 8:
        w0chunks = 0  # dense fallback: no split for scale 0
    diag0 = np.zeros((2, NPAD, 128), dtype=np.float32)
    for i in range(4):
        for d in range(2):
            m = adjs[i] if d == 0 else adjs[i].T
            a = m + eye
            a = a / a.sum(1, keepdims=True)
            if i == 0 and w0chunks:
                dv = np.diag(a).copy()
                a = a.copy()
                np.fill_diagonal(a, 0.0)   # exact split: A = D + S
                for v in range(NNODE):
                    diag0[d, v, v % 128] = dv[v]
            adjt[i * 2 + d, :NNODE, :NNODE] = a.T
    g_list, beta_list = _composite_convs(inputs)
    al, be = PROPALPHA, 1.0 - PROPALPHA
    gms = {}   # (i, d, j, half) -> [121, F_i] fp32
    for i in range(4):
        for d in range(2):
            wm = np.asarray(inputs["mp1_w" if d == 0 else "mp2_w"][i],
                            dtype=np.float64).reshape(C, 3 * C)
            bm = np.asarray(inputs["mp1_b" if d == 0 else "mp2_b"][i],
                            dtype=np.float64)
            W0, W1, W2 = wm[:, :C], wm[:, C:2 * C], wm[:, 2 * C:]
            E = [W0 + al * (W1 + W2), be * W1 + al * be * W2, (be ** 2) * W2]
            bias = [E[0] @ beta_list[i] + bm, E[1] @ beta_list[i],
                    E[2] @ beta_list[i]]
            for j in range(3):
                gE = E[j] @ g_list[i]
                for half in range(2):
                    gms[(i, d, j, half)] = _band_matrix(gE, bias[j], i, half)
    # scale0: plain conv, kernel length T, bias row
    g00 = np.zeros((T + 1, C), dtype=np.float64)
    g00[:T, :] = np.asarray(inputs["scale0_w"], dtype=np.float64)[:, 0, 0, :].T
    g00[T, :] = np.asarray(inputs["scale0_b"], dtype=np.float64)
    g00 = g00.astype(np.float32)
    return adjt, gms, g00, diag0, w0chunks


def _setup_ntff_hook():
    """Enable NTFF profiling under axon (used only when FPN_TRACE=1)."""
    import types
    if "antenv.axon_hooks" not in sys.modules:
        ah = types.ModuleType("antenv.axon_hooks")
        ah._hook = None
        ah.set_axon_ntff_profile_hook = lambda h: setattr(ah, "_hook", h)
        ah.get_axon_ntff_profile_hook = lambda: ah._hook
        sys.modules["antenv.axon_hooks"] = ah
    ah = sys.modules["antenv.axon_hooks"]
    if getattr(ah, "_hook", None) is None:
        try:
            from trn_agent_boot.trn_boot import _ntff_profile_via_ctypes
            ah.set_axon_ntff_profile_hook(
                _ntff_profile_via_ctypes('/opt/axon/libaxon_pjrt.so'))
        except Exception as e:
            print("ntff hook setup failed:", e, file=sys.stderr)


_NC = {}


def _build_nc(w0chunks):
    if w0chunks in _NC:
        return _NC[w0chunks]
    import concourse.bacc as bacc
    import concourse.mybir as mybir
    from concourse.tile import TileContext

    f32 = mybir.dt.float32
    f32r = mybir.dt.float32r

    nc = bacc.Bacc(None, target_bir_lowering=False)
    xa_d = nc.dram_tensor("xa", (T + 1, NPAD), f32, kind="ExternalInput")
    adjt_d = nc.dram_tensor("adjt", (8, NPAD, NPAD), f32, kind="ExternalInput")
    diag0_d = nc.dram_tensor("diag0", (2, NPAD, 128), f32, kind="ExternalInput")
    g00_d = nc.dram_tensor("g00", (T + 1, C), f32, kind="ExternalInput")
    gm_d = {}
    for i in range(4):
        for d in range(2):
            for j in range(3):
                gm_d[(i, d, j)] = nc.dram_tensor(
                    f"gm_{i}_{d}_{j}", (T + 1, F_LIST[i]), f32,
                    kind="ExternalInput")
    out_d = {}
    for i in range(4):
        for d in range(2):
            out_d[(i, d)] = nc.dram_tensor(
                f"out_{i}_{d}", (NPAD, F_LIST[i]), f32, kind="ExternalOutput")
    out00_d = nc.dram_tensor("out00", (NPAD, C), f32, kind="ExternalOutput")

    FMAX = FCH[0]  # 432

    with TileContext(nc) as tc:
        with tc.tile_pool(name="const", bufs=1) as constp, \
             tc.tile_pool(name="stage", bufs=2) as stage, \
             tc.tile_pool(name="adj", bufs=2) as adjp, \
             tc.tile_pool(name="gp", bufs=2) as gp, \
             tc.tile_pool(name="sp", bufs=2) as sp, \
             tc.tile_pool(name="op", bufs=3) as op, \
             tc.tile_pool(name="ps", bufs=2, space="PSUM") as ps, \
             tc.tile_pool(name="psc3", bufs=3, space="PSUM") as psc3:

            xa_f = constp.tile([T + 1, NPAD], f32, name="xa_f")
            nc.gpsimd.dma_start(xa_f[:], xa_d[:, :])
            xa_r = constp.tile([T + 1, NPAD], f32r, name="xa_r")
            nc.vector.tensor_copy(xa_r[:], xa_f[:])

            # scale0 first: tiny matmuls warm the PE while adjacencies stream
            g00s = stage.tile([T + 1, C], f32, tag="gstage", name="g00s")
            nc.gpsimd.dma_start(g00s[:], g00_d[:, :])
            g00r = constp.tile([T + 1, C], f32r, name="g00r")
            nc.vector.tensor_copy(g00r[:], g00s[:])
            for w in range(8):
                p0 = psc3.tile([128, C], f32, tag="psc", name=f"p0_{w}")
                nc.tensor.matmul(p0[:], xa_r[:, w * 128:(w + 1) * 128],
                                 g00r[:], start=True, stop=True)
                o0 = op.tile([128, C], f32, tag="o0", name=f"o0_{w}")
                nc.vector.tensor_copy(o0[:], p0[:])
                nc.sync.dma_start(out00_d[w * 128:(w + 1) * 128, :], o0[:])

            for i in range(4):
                Fi, fc = F_LIST[i], FCH[i]
                compact = (i == 0 and w0chunks > 0)
                for d in range(2):
                    kidx = i * 2 + d
                    gr = []
                    for j in range(3):
                        gs = stage.tile([T + 1, Fi], f32, tag="gstage",
                                        name=f"gst_{i}_{d}_{j}")
                        nc.gpsimd.dma_start(gs[:], gm_d[(i, d, j)][:, :])
                        gj = gp.tile([T + 1, F_LIST[0]], f32r, tag=f"gr{j}",
                                     name=f"gr_{i}_{d}_{j}")[:, :Fi]
                        nc.scalar.copy(gj, gs[:])
                        gr.append(gj)
                    # which stationary w-chunks of S^T are needed, and which
                    # output v-chunks have S contributions
                    if compact and d == 0:
                        wlist = list(range(w0chunks))   # S columns compacted
                        vfull = list(range(8))
                    elif compact and d == 1:
                        wlist = list(range(8))
                        vfull = list(range(w0chunks))   # S rows compacted
                    else:
                        wlist = list(range(8))
                        vfull = list(range(8))
                    adjr = {}
                    for w in wlist:
                        csl = (slice(0, w0chunks * 128) if (compact and d == 1)
                               else slice(0, NPAD))
                        cw = csl.stop - csl.start
                        st = stage.tile([128, NPAD], f32, tag="adjstage",
                                        name=f"ast_{i}_{d}_{w}")[:, :cw]
                        nc.sync.dma_start(
                            st, adjt_d[kidx, w * 128:(w + 1) * 128, csl])
                        ar = adjp.tile([128, NPAD], f32r, tag=f"adjr{w}",
                                       name=f"adjr_{i}_{d}_{w}")[:, :cw]
                        nc.scalar.copy(ar, st)
                        adjr[w] = ar
                    dgr = {}
                    if compact:
                        for v in range(8):
                            dst = stage.tile([128, 128], f32, tag="dstage",
                                             name=f"dst_{d}_{v}")
                            nc.sync.dma_start(
                                dst[:], diag0_d[d, v * 128:(v + 1) * 128, :])
                            dg = adjp.tile([128, 128], f32r, tag=f"dg{v}",
                                           name=f"dg_{d}_{v}")
                            nc.scalar.copy(dg[:], dst[:])
                            dgr[v] = dg

                    def hop(v, g_band, rhs_tiles, psname, f, sl):
                        """One output accumulation group for chunk v."""
                        vs = slice(v * 128, (v + 1) * 128)
                        mm = [("conv", None)]
                        if compact:
                            mm.append(("diag", None))
                            if (d == 0) or (v in vfull):
                                mm += [("adj", w) for w in wlist]
                        else:
                            mm += [("adj", w) for w in wlist]
                        ph = ps.tile([128, FMAX], f32, tag=psname,
                                     name=f"{psname}_{i}_{d}_{f}_{v}")[:, :fc]
                        last = len(mm) - 1
                        for k, (kind, w) in enumerate(mm):
                            if kind == "conv":
                                nc.tensor.matmul(ph, xa_r[:, vs],
                                                 g_band[:, sl],
                                                 start=(k == 0),
                                                 stop=(k == last))
                            elif kind == "diag":
                                nc.tensor.matmul(ph, dgr[v][:],
                                                 rhs_tiles[v],
                                                 start=(k == 0),
                                                 stop=(k == last))
                            else:
                                lh = (adjr[w][:, vs] if not (compact and d == 1)
                                      else adjr[w][:, v * 128:(v + 1) * 128])
                                nc.tensor.matmul(ph, lh, rhs_tiles[w],
                                                 start=(k == 0),
                                                 stop=(k == last))
                        return ph

                    for f in range(4):
                        sl = slice(f * fc, (f + 1) * fc)
                        s2t = []
                        for w in range(8):
                            pc = psc3.tile([128, FMAX], f32, tag="psc",
                                           name=f"pc_{i}_{d}_{f}_{w}")[:, :fc]
                            nc.tensor.matmul(
                                pc, xa_r[:, w * 128:(w + 1) * 128],
                                gr[2][:, sl], start=True, stop=True)
                            s2 = sp.tile([128, FMAX], f32r, tag=f"s2_{w}",
                                         name=f"s2_{i}_{d}_{f}_{w}")[:, :fc]
                            nc.vector.tensor_copy(s2, pc)
                            s2t.append(s2)
                        ur = []
                        for v in range(8):
                            ph = hop(v, gr[1], s2t, "ps1", f, sl)
                            u = sp.tile([128, FMAX], f32r, tag=f"u_{v}",
                                        name=f"u_{i}_{d}_{f}_{v}")[:, :fc]
                            nc.vector.tensor_copy(u, ph)
                            ur.append(u)
                        for v in range(8):
                            ph2 = hop(v, gr[0], ur, "ps2", f, sl)
                            ob = op.tile([128, FMAX], f32, tag="ob",
                                         name=f"ob_{i}_{d}_{f}_{v}")[:, :fc]
                            nc.vector.tensor_copy(ob, ph2)
                            nc.sync.dma_start(
                                out_d[(i, d)][v * 128:(v + 1) * 128, sl], ob)
    nc.compile()
    _NC[w0chunks] = nc
    return nc


def kernel(**inputs):
    inputs = {k: np.asarray(v) for k, v in inputs.items()}
    adjt, gms, g00, diag0, w0chunks = _host_prep(inputs)
    x = inputs["x"].astype(np.float32)  # [B, T, N]

    nc = _build_nc(w0chunks)
    in_maps = []
    for b in range(B):
        for half in range(2):
            xa = np.zeros((T + 1, NPAD), dtype=np.float32)
            xa[:T, :NNODE] = x[b]
            xa[T, :NNODE] = 1.0
            m = {"xa": xa, "adjt": adjt, "g00": g00, "diag0": diag0}
            for i in range(4):
                for d in range(2):
                    for j in range(3):
                        m[f"gm_{i}_{d}_{j}"] = gms[(i, d, j, half)]
            in_maps.append(m)

    from concourse.bass_utils import run_bass_kernel_spmd
    trace = bool(os.environ.get("FPN_TRACE"))
    if trace:
        _setup_ntff_hook()
    r = run_bass_kernel_spmd(nc, in_maps, core_ids=list(range(8)),
                             trace=trace)
    kernel.last_exec_ns = r.exec_time_ns
    kernel.last_insts = r.instructions_and_trace
    res = r.results

    outs = []
    s0 = np.empty((B, C, NNODE, 1), dtype=np.float32)
    for b in range(B):
        s0[b, :, :, 0] = res[b * 2]["out00"][:NNODE, :].T
    outs.append(s0)
    for i in range(4):
        Li, Lhi = L_LIST[i], LH[i]
        full = np.empty((B, C, NNODE, Li), dtype=np.float32)
        for b in range(B):
            for half in range(2):
                r = res[b * 2 + half]
                part = r[f"out_{i}_0"][:NNODE] + r[f"out_{i}_1"][:NNODE]
                # [N, Lh*32] -> [C, N, Lh]
                part = part.reshape(NNODE, Lhi, C).transpose(2, 0, 1)
                if half == 0:
                    full[b, :, :, :Lhi] = part
                else:
                    j0 = 2 * Lhi - Li
                    full[b, :, :, Lhi:] = part[:, :, j0:]
        outs.append(full)
    return tuple(outs)


# revision 14
# speedup vs baseline: 1.0317x; 1.0317x over previous
"""FPN encoder (MTGNN/MAGNN-style) Trainium2 kernel.

Strategy:
 - Host: graph constructor (tiny, input-only-dependent, exact-tie-sensitive
   top-k) computed bit-exactly with jax-CPU in a subprocess; adjacencies are
   then row-normalized, transposed, padded to 1024 and replicated to all
   cores like weights. The linear FPN conv pyramid and the mixprop 1x1-conv
   channel mixes are folded (host-side weight preprocessing) into per-scale
   band matrices G0/G1/G2 of one composite temporal conv each, with biases as
   an extra constant-one row of x.
 - Device (8 cores, zero collectives): core (b, h) handles batch b and the
   h-th half of the time axis of every scale. Everything is node-major
   matmuls on the tensor engine in float32r:
       S2 = xa^T G2 ; u = A (S2) + xa^T G1 ; out_d = A u + xa^T G0
   with A-chunks as stationary operands, two hops per scale per direction.
 - Host gather: slice away node padding, fold the two directions' partial
   sums, reassemble the 5-tuple of full outputs.
"""

import os
import sys
import subprocess
import tempfile

import numpy as np

for _p in ('/opt/trn_rl_repo', '/root/.axon_site/_ro/trn_rl_repo'):
    if os.path.isdir(_p) and _p not in sys.path:
        sys.path.append(_p)

B, T, NNODE, C, D = 4, 120, 1000, 32, 40
KERNEL_SET = [14, 7, 6, 3]
PROPALPHA, ALPHA, TOPK = 0.05, 3.0, 20
SCALE_SET = [1.0, 0.8, 0.6, 0.5]
NPAD = 1024
L_LIST = [107, 101, 96, 94]
LH = [54, 51, 48, 47]          # per-scale half lengths (ceil(L/2))
F_LIST = [32 * l for l in LH]  # free width per scale: 1728,1632,1536,1504
FCH = [f // 4 for f in F_LIST]  # free chunk: 432,408,384,376 (256..512)
GLEN = [14, 20, 25, 27]        # composite conv kernel lengths

_CHILD = r'''
import sys, numpy as np
import jax, jax.numpy as jnp
pin, pout = sys.argv[1], sys.argv[2]
z = np.load(pin)
emb1, emb2 = jnp.asarray(z["emb1"]), jnp.asarray(z["emb2"])
l1w, l1b = jnp.asarray(z["lin1_w"]), jnp.asarray(z["lin1_b"])
l2w, l2b = jnp.asarray(z["lin2_w"]), jnp.asarray(z["lin2_b"])
NN, KK, AL = 1000, 20, 3.0
SC = [1.0, 0.8, 0.6, 0.5]
n1, n2 = emb1, emb2
rows = jnp.arange(NN)[:, None]
adjs = []
for i in range(4):
    n1 = jnp.tanh(AL * ((n1 * SC[i]) @ l1w[i].T + l1b[i]))
    n2 = jnp.tanh(AL * ((n2 * SC[i]) @ l2w[i].T + l2b[i]))
    a = n1 @ n2.T - n2 @ n1.T
    adj0 = jax.nn.relu(jnp.tanh(AL * a))
    _, t1 = jax.lax.top_k(adj0, KK)
    mask = jnp.zeros_like(adj0).at[rows, t1].set(1.0)
    adjs.append(np.asarray(adj0 * mask))
np.savez(pout, **{f"a{i}": adjs[i] for i in range(4)})
'''


def _graph_adjs(inputs):
    """Replicate reference.graph_construct bit-exactly on jax-CPU."""
    with tempfile.TemporaryDirectory() as td:
        pin = os.path.join(td, "in.npz")
        pout = os.path.join(td, "out.npz")
        np.savez(pin, emb1=inputs["emb1"], emb2=inputs["emb2"],
                 lin1_w=inputs["lin1_w"], lin1_b=inputs["lin1_b"],
                 lin2_w=inputs["lin2_w"], lin2_b=inputs["lin2_b"])
        env = dict(os.environ)
        env["JAX_PLATFORMS"] = "cpu"
        env.pop("TRN_TERMINAL_POOL_IPS", None)
        parts = []
        for chunk in (env.get("PYTHONPATH", ""), env.get("NIX_PYTHONPATH", "")):
            parts.extend(p for p in chunk.split(os.pathsep) if p)
        parts.extend(p for p in sys.path if p)
        env["PYTHONPATH"] = os.pathsep.join(dict.fromkeys(parts))
        r = subprocess.run([sys.executable, "-c", _CHILD, pin, pout],
                           env=env, capture_output=True, text=True, timeout=900)
        if r.returncode != 0:
            raise RuntimeError(f"graph subprocess failed:\n{r.stdout}\n{r.stderr}")
        z = np.load(pout)
        return [z[f"a{i}"] for i in range(4)]


def _composite_convs(inputs):
    """Compose the linear FPN pyramid into one temporal kernel per scale.

    Returns g_list[i] [32, GLEN[i]] and beta_list[i] [32] (fp64) such that
    scales[i][c, n, l] = sum_k g[c, k] x[l + k, n] + beta[c].
    """
    g_list, beta_list = [], []
    cur_g = None
    cur_beta = None
    for idx, k in enumerate(KERNEL_SET):
        w = np.asarray(inputs[f"msc_w{idx}"], dtype=np.float64)
        b = np.asarray(inputs[f"msc_b{idx}"], dtype=np.float64)
        if idx == 0:
            cur_g = w[:, 0, 0, :].copy()
            cur_beta = b.copy()
        else:
            kp = cur_g.shape[1]
            ng = np.zeros((C, kp + k - 1))
            for t2 in range(k):
                ng[:, t2:t2 + kp] += w[:, :, 0, t2] @ cur_g
            cur_beta = b + w[:, :, 0, :].sum(-1) @ cur_beta
            cur_g = ng
        g_list.append(cur_g.copy())
        beta_list.append(cur_beta.copy())
    return g_list, beta_list


def _band_matrix(gE, bE, scale, half):
    """[121, 32*LH] band matrix for one (scale, set, half): columns are
    (l_local, c); rows are input time (120) plus the constant-one bias row."""
    Ki = gE.shape[1]
    Lh = LH[scale]
    l0 = 0 if half == 0 else L_LIST[scale] - Lh
    Gm = np.zeros((T + 1, 32 * Lh), dtype=np.float64)
    gT = gE.T  # [Ki, 32]
    for ll in range(Lh):
        l = l0 + ll
        Gm[l:l + Ki, ll * 32:(ll + 1) * 32] = gT
    Gm[T, :] = np.tile(bE, Lh)
    return Gm.astype(np.float32)


def _host_prep(inputs):
    """All host-side preprocessing -> per-core input maps (minus xa)."""
    adjs = _graph_adjs(inputs)
    eye = np.eye(NNODE, dtype=np.float32)
    adjt = np.zeros((8, NPAD, NPAD), dtype=np.float32)
    # scale-0 compaction: the saturated-plateau top-k picks low column
    # indices, so the off-diagonal part of A_0 spans few node chunks.
    used = (adjs[0] > 0).any(0)
    ncol0 = int(np.nonzero(used)[0].max()) + 1 if used.any() else 1
    w0chunks = (ncol0 + 127) // 128
    if w0chunks >= 8:
        w0chunks = 0  # dense fallback: no split for scale 0
    diag0 = np.zeros((2, NPAD, 128), dtype=np.float32)
    for i in range(4):
        for d in range(2):
            m = adjs[i] if d == 0 else adjs[i].T
            a = m + eye
            a = a / a.sum(1, keepdims=True)
            if i == 0 and w0chunks:
                dv = np.diag(a).copy()
                a = a.copy()
                np.fill_diagonal(a, 0.0)   # exact split: A = D + S
                for v in range(NNODE):
                    diag0[d, v, v % 128] = dv[v]
            adjt[i * 2 + d, :NNODE, :NNODE] = a.T
    g_list, beta_list = _composite_convs(inputs)
    al, be = PROPALPHA, 1.0 - PROPALPHA
    gms = {}   # (i, d, j, half) -> [121, F_i] fp32
    for i in range(4):
        for d in range(2):
            wm = np.asarray(inputs["mp1_w" if d == 0 else "mp2_w"][i],
                            dtype=np.float64).reshape(C, 3 * C)
            bm = np.asarray(inputs["mp1_b" if d == 0 else "mp2_b"][i],
                            dtype=np.float64)
            W0, W1, W2 = wm[:, :C], wm[:, C:2 * C], wm[:, 2 * C:]
            E = [W0 + al * (W1 + W2), be * W1 + al * be * W2, (be ** 2) * W2]
            bias = [E[0] @ beta_list[i] + bm, E[1] @ beta_list[i],
                    E[2] @ beta_list[i]]
            for j in range(3):
                gE = E[j] @ g_list[i]
                for half in range(2):
                    gms[(i, d, j, half)] = _band_matrix(gE, bias[j], i, half)
    # scale0: plain conv, kernel length T, bias row
    g00 = np.zeros((T + 1, C), dtype=np.float64)
    g00[:T, :] = np.asarray(inputs["scale0_w"], dtype=np.float64)[:, 0, 0, :].T
    g00[T, :] = np.asarray(inputs["scale0_b"], dtype=np.float64)
    g00 = g00.astype(np.float32)
    return adjt, gms, g00, diag0, w0chunks


def _setup_ntff_hook():
    """Enable NTFF profiling under axon (used only when FPN_TRACE=1)."""
    import types
    if "antenv.axon_hooks" not in sys.modules:
        ah = types.ModuleType("antenv.axon_hooks")
        ah._hook = None
        ah.set_axon_ntff_profile_hook = lambda h: setattr(ah, "_hook", h)
        ah.get_axon_ntff_profile_hook = lambda: ah._hook
        sys.modules["antenv.axon_hooks"] = ah
    ah = sys.modules["antenv.axon_hooks"]
    if getattr(ah, "_hook", None) is None:
        try:
            from trn_agent_boot.trn_boot import _ntff_profile_via_ctypes
            ah.set_axon_ntff_profile_hook(
                _ntff_profile_via_ctypes('/opt/axon/libaxon_pjrt.so'))
        except Exception as e:
            print("ntff hook setup failed:", e, file=sys.stderr)


_NC = {}


def _build_nc(w0chunks):
    if w0chunks in _NC:
        return _NC[w0chunks]
    import concourse.bacc as bacc
    import concourse.mybir as mybir
    from concourse.tile import TileContext

    f32 = mybir.dt.float32
    f32r = mybir.dt.float32r

    nc = bacc.Bacc(None, target_bir_lowering=False)
    xa_d = nc.dram_tensor("xa", (T + 1, NPAD), f32, kind="ExternalInput")
    adjt_d = nc.dram_tensor("adjt", (8, NPAD, NPAD), f32, kind="ExternalInput")
    diag0_d = nc.dram_tensor("diag0", (2, NPAD, 128), f32, kind="ExternalInput")
    g00_d = nc.dram_tensor("g00", (T + 1, C), f32, kind="ExternalInput")
    gm_d = {}
    for i in range(4):
        for d in range(2):
            for j in range(3):
                gm_d[(i, d, j)] = nc.dram_tensor(
                    f"gm_{i}_{d}_{j}", (T + 1, F_LIST[i]), f32,
                    kind="ExternalInput")
    out_d = {}
    for i in range(4):
        for d in range(2):
            out_d[(i, d)] = nc.dram_tensor(
                f"out_{i}_{d}", (NPAD, F_LIST[i]), f32, kind="ExternalOutput")
    out00_d = nc.dram_tensor("out00", (NPAD, C), f32, kind="ExternalOutput")

    FMAX = FCH[0]  # 432

    with TileContext(nc) as tc:
        with tc.tile_pool(name="const", bufs=1) as constp, \
             tc.tile_pool(name="stage", bufs=2) as stage, \
             tc.tile_pool(name="adj", bufs=2) as adjp, \
             tc.tile_pool(name="gp", bufs=2) as gp, \
             tc.tile_pool(name="sp", bufs=2) as sp, \
             tc.tile_pool(name="op", bufs=3) as op, \
             tc.tile_pool(name="ps", bufs=2, space="PSUM") as ps, \
             tc.tile_pool(name="psc3", bufs=3, space="PSUM") as psc3:

            xa_f = constp.tile([T + 1, NPAD], f32, name="xa_f")
            nc.gpsimd.dma_start(xa_f[:], xa_d[:, :])
            xa_r = constp.tile([T + 1, NPAD], f32r, name="xa_r")
            nc.vector.tensor_copy(xa_r[:], xa_f[:])

            # scale0 first: tiny matmuls warm the PE while adjacencies stream
            g00s = stage.tile([T + 1, C], f32, tag="gstage", name="g00s")
            nc.gpsimd.dma_start(g00s[:], g00_d[:, :])
            g00r = constp.tile([T + 1, C], f32r, name="g00r")
            nc.vector.tensor_copy(g00r[:], g00s[:])
            for w in range(8):
                p0 = psc3.tile([128, C], f32, tag="psc", name=f"p0_{w}")
                nc.tensor.matmul(p0[:], xa_r[:, w * 128:(w + 1) * 128],
                                 g00r[:], start=True, stop=True)
                o0 = op.tile([128, C], f32, tag="o0", name=f"o0_{w}")
                nc.vector.tensor_copy(o0[:], p0[:])
                nc.sync.dma_start(out00_d[w * 128:(w + 1) * 128, :], o0[:])

            for i in range(4):
                Fi, fc = F_LIST[i], FCH[i]
                compact = (i == 0 and w0chunks > 0)
                for d in range(2):
                    kidx = i * 2 + d
                    gr = []
                    for j in range(3):
                        gs = stage.tile([T + 1, Fi], f32, tag="gstage",
                                        name=f"gst_{i}_{d}_{j}")
                        nc.gpsimd.dma_start(gs[:], gm_d[(i, d, j)][:, :])
                        gj = gp.tile([T + 1, F_LIST[0]], f32r, tag=f"gr{j}",
                                     name=f"gr_{i}_{d}_{j}")[:, :Fi]
                        nc.scalar.copy(gj, gs[:])
                        gr.append(gj)
                    # which stationary w-chunks of S^T are needed, and which
                    # output v-chunks have S contributions
                    if compact and d == 0:
                        wlist = list(range(w0chunks))   # S columns compacted
                        vfull = list(range(8))
                    elif compact and d == 1:
                        wlist = list(range(8))
                        vfull = list(range(w0chunks))   # S rows compacted
                    else:
                        wlist = list(range(8))
                        vfull = list(range(8))
                    adjr = {}
                    for w in wlist:
                        csl = (slice(0, w0chunks * 128) if (compact and d == 1)
                               else slice(0, NPAD))
                        cw = csl.stop - csl.start
                        st = stage.tile([128, NPAD], f32, tag="adjstage",
                                        name=f"ast_{i}_{d}_{w}")[:, :cw]
                        nc.gpsimd.dma_start(
                            st, adjt_d[kidx, w * 128:(w + 1) * 128, csl])
                        ar = adjp.tile([128, NPAD], f32r, tag=f"adjr{w}",
                                       name=f"adjr_{i}_{d}_{w}")[:, :cw]
                        nc.scalar.copy(ar, st)
                        adjr[w] = ar
                    dgr = {}
                    if compact:
                        for v in range(8):
                            dst = stage.tile([128, 128], f32, tag="dstage",
                                             name=f"dst_{d}_{v}")
                            nc.gpsimd.dma_start(
                                dst[:], diag0_d[d, v * 128:(v + 1) * 128, :])
                            dg = adjp.tile([128, 128], f32r, tag=f"dg{v}",
                                           name=f"dg_{d}_{v}")
                            nc.scalar.copy(dg[:], dst[:])
                            dgr[v] = dg

                    def hop(v, g_band, rhs_tiles, psname, f, sl):
                        """One output accumulation group for chunk v."""
                        vs = slice(v * 128, (v + 1) * 128)
                        mm = [("conv", None)]
                        if compact:
                            mm.append(("diag", None))
                            if (d == 0) or (v in vfull):
                                mm += [("adj", w) for w in wlist]
                        else:
                            mm += [("adj", w) for w in wlist]
                        ph = ps.tile([128, FMAX], f32, tag=psname,
                                     name=f"{psname}_{i}_{d}_{f}_{v}")[:, :fc]
                        last = len(mm) - 1
                        for k, (kind, w) in enumerate(mm):
                            if kind == "conv":
                                nc.tensor.matmul(ph, xa_r[:, vs],
                                                 g_band[:, sl],
                                                 start=(k == 0),
                                                 stop=(k == last))
                            elif kind == "diag":
                                nc.tensor.matmul(ph, dgr[v][:],
                                                 rhs_tiles[v],
                                                 start=(k == 0),
                                                 stop=(k == last))
                            else:
                                lh = (adjr[w][:, vs] if not (compact and d == 1)
                                      else adjr[w][:, v * 128:(v + 1) * 128])
                                nc.tensor.matmul(ph, lh, rhs_tiles[w],
                                                 start=(k == 0),
                                                 stop=(k == last))
                        return ph

                    for f in range(4):
                        sl = slice(f * fc, (f + 1) * fc)
                        s2t = []
                        for w in range(8):
                            pc = psc3.tile([128, FMAX], f32, tag="psc",
                                           name=f"pc_{i}_{d}_{f}_{w}")[:, :fc]
                            nc.tensor.matmul(
                                pc, xa_r[:, w * 128:(w + 1) * 128],
                                gr[2][:, sl], start=True, stop=True)
                            s2 = sp.tile([128, FMAX], f32r, tag=f"s2_{w}",
                                         name=f"s2_{i}_{d}_{f}_{w}")[:, :fc]
                            nc.vector.tensor_copy(s2, pc)
                            s2t.append(s2)
                        ur = []
                        for v in range(8):
                            ph = hop(v, gr[1], s2t, "ps1", f, sl)
                            u = sp.tile([128, FMAX], f32r, tag=f"u_{v}",
                                        name=f"u_{i}_{d}_{f}_{v}")[:, :fc]
                            nc.vector.tensor_copy(u, ph)
                            ur.append(u)
                        for v in range(8):
                            ph2 = hop(v, gr[0], ur, "ps2", f, sl)
                            ob = op.tile([128, FMAX], f32, tag="ob",
                                         name=f"ob_{i}_{d}_{f}_{v}")[:, :fc]
                            nc.vector.tensor_copy(ob, ph2)
                            nc.sync.dma_start(
                                out_d[(i, d)][v * 128:(v + 1) * 128, sl], ob)
    nc.compile()
    _NC[w0chunks] = nc
    return nc


def kernel(**inputs):
    inputs = {k: np.asarray(v) for k, v in inputs.items()}
    adjt, gms, g00, diag0, w0chunks = _host_prep(inputs)
    x = inputs["x"].astype(np.float32)  # [B, T, N]

    nc = _build_nc(w0chunks)
    in_maps = []
    for b in range(B):
        for half in range(2):
            xa = np.zeros((T + 1, NPAD), dtype=np.float32)
            xa[:T, :NNODE] = x[b]
            xa[T, :NNODE] = 1.0
            m = {"xa": xa, "adjt": adjt, "g00": g00, "diag0": diag0}
            for i in range(4):
                for d in range(2):
                    for j in range(3):
                        m[f"gm_{i}_{d}_{j}"] = gms[(i, d, j, half)]
            in_maps.append(m)

    from concourse.bass_utils import run_bass_kernel_spmd
    trace = bool(os.environ.get("FPN_TRACE"))
    if trace:
        _setup_ntff_hook()
    r = run_bass_kernel_spmd(nc, in_maps, core_ids=list(range(8)),
                             trace=trace)
    kernel.last_exec_ns = r.exec_time_ns
    kernel.last_insts = r.instructions_and_trace
    res = r.results

    outs = []
    s0 = np.empty((B, C, NNODE, 1), dtype=np.float32)
    for b in range(B):
        s0[b, :, :, 0] = res[b * 2]["out00"][:NNODE, :].T
    outs.append(s0)
    for i in range(4):
        Li, Lhi = L_LIST[i], LH[i]
        full = np.empty((B, C, NNODE, Li), dtype=np.float32)
        for b in range(B):
            for half in range(2):
                r = res[b * 2 + half]
                part = r[f"out_{i}_0"][:NNODE] + r[f"out_{i}_1"][:NNODE]
                # [N, Lh*32] -> [C, N, Lh]
                part = part.reshape(NNODE, Lhi, C).transpose(2, 0, 1)
                if half == 0:
                    full[b, :, :, :Lhi] = part
                else:
                    j0 = 2 * Lhi - Li
                    full[b, :, :, Lhi:] = part[:, :, j0:]
        outs.append(full)
    return tuple(outs)


# revision 15
# speedup vs baseline: 1.0513x; 1.0190x over previous
"""FPN encoder (MTGNN/MAGNN-style) Trainium2 kernel.

Strategy:
 - Host: graph constructor (tiny, input-only-dependent, exact-tie-sensitive
   top-k) computed bit-exactly with jax-CPU in a subprocess; adjacencies are
   then row-normalized, transposed, padded to 1024 and replicated to all
   cores like weights. The linear FPN conv pyramid and the mixprop 1x1-conv
   channel mixes are folded (host-side weight preprocessing) into per-scale
   band matrices G0/G1/G2 of one composite temporal conv each, with biases as
   an extra constant-one row of x.
 - Device (8 cores, zero collectives): core (b, h) handles batch b and the
   h-th half of the time axis of every scale. Everything is node-major
   matmuls on the tensor engine in float32r:
       S2 = xa^T G2 ; u = A (S2) + xa^T G1 ; out_d = A u + xa^T G0
   with A-chunks as stationary operands, two hops per scale per direction.
 - Host gather: slice away node padding, fold the two directions' partial
   sums, reassemble the 5-tuple of full outputs.
"""

import os
import sys
import subprocess
import tempfile

import numpy as np

for _p in ('/opt/trn_rl_repo', '/root/.axon_site/_ro/trn_rl_repo'):
    if os.path.isdir(_p) and _p not in sys.path:
        sys.path.append(_p)

B, T, NNODE, C, D = 4, 120, 1000, 32, 40
KERNEL_SET = [14, 7, 6, 3]
PROPALPHA, ALPHA, TOPK = 0.05, 3.0, 20
SCALE_SET = [1.0, 0.8, 0.6, 0.5]
NPAD = 1024
L_LIST = [107, 101, 96, 94]
LH = [54, 51, 48, 47]          # per-scale half lengths (ceil(L/2))
F_LIST = [32 * l for l in LH]  # free width per scale: 1728,1632,1536,1504
FCH = [f // 4 for f in F_LIST]  # free chunk: 432,408,384,376 (256..512)
GLEN = [14, 20, 25, 27]        # composite conv kernel lengths

_CHILD = r'''
import sys, numpy as np
import jax, jax.numpy as jnp
pin, pout = sys.argv[1], sys.argv[2]
z = np.load(pin)
emb1, emb2 = jnp.asarray(z["emb1"]), jnp.asarray(z["emb2"])
l1w, l1b = jnp.asarray(z["lin1_w"]), jnp.asarray(z["lin1_b"])
l2w, l2b = jnp.asarray(z["lin2_w"]), jnp.asarray(z["lin2_b"])
NN, KK, AL = 1000, 20, 3.0
SC = [1.0, 0.8, 0.6, 0.5]
n1, n2 = emb1, emb2
rows = jnp.arange(NN)[:, None]
adjs = []
for i in range(4):
    n1 = jnp.tanh(AL * ((n1 * SC[i]) @ l1w[i].T + l1b[i]))
    n2 = jnp.tanh(AL * ((n2 * SC[i]) @ l2w[i].T + l2b[i]))
    a = n1 @ n2.T - n2 @ n1.T
    adj0 = jax.nn.relu(jnp.tanh(AL * a))
    _, t1 = jax.lax.top_k(adj0, KK)
    mask = jnp.zeros_like(adj0).at[rows, t1].set(1.0)
    adjs.append(np.asarray(adj0 * mask))
np.savez(pout, **{f"a{i}": adjs[i] for i in range(4)})
'''


def _graph_adjs(inputs):
    """Replicate reference.graph_construct bit-exactly on jax-CPU."""
    with tempfile.TemporaryDirectory() as td:
        pin = os.path.join(td, "in.npz")
        pout = os.path.join(td, "out.npz")
        np.savez(pin, emb1=inputs["emb1"], emb2=inputs["emb2"],
                 lin1_w=inputs["lin1_w"], lin1_b=inputs["lin1_b"],
                 lin2_w=inputs["lin2_w"], lin2_b=inputs["lin2_b"])
        env = dict(os.environ)
        env["JAX_PLATFORMS"] = "cpu"
        env.pop("TRN_TERMINAL_POOL_IPS", None)
        parts = []
        for chunk in (env.get("PYTHONPATH", ""), env.get("NIX_PYTHONPATH", "")):
            parts.extend(p for p in chunk.split(os.pathsep) if p)
        parts.extend(p for p in sys.path if p)
        env["PYTHONPATH"] = os.pathsep.join(dict.fromkeys(parts))
        r = subprocess.run([sys.executable, "-c", _CHILD, pin, pout],
                           env=env, capture_output=True, text=True, timeout=900)
        if r.returncode != 0:
            raise RuntimeError(f"graph subprocess failed:\n{r.stdout}\n{r.stderr}")
        z = np.load(pout)
        return [z[f"a{i}"] for i in range(4)]


def _composite_convs(inputs):
    """Compose the linear FPN pyramid into one temporal kernel per scale.

    Returns g_list[i] [32, GLEN[i]] and beta_list[i] [32] (fp64) such that
    scales[i][c, n, l] = sum_k g[c, k] x[l + k, n] + beta[c].
    """
    g_list, beta_list = [], []
    cur_g = None
    cur_beta = None
    for idx, k in enumerate(KERNEL_SET):
        w = np.asarray(inputs[f"msc_w{idx}"], dtype=np.float64)
        b = np.asarray(inputs[f"msc_b{idx}"], dtype=np.float64)
        if idx == 0:
            cur_g = w[:, 0, 0, :].copy()
            cur_beta = b.copy()
        else:
            kp = cur_g.shape[1]
            ng = np.zeros((C, kp + k - 1))
            for t2 in range(k):
                ng[:, t2:t2 + kp] += w[:, :, 0, t2] @ cur_g
            cur_beta = b + w[:, :, 0, :].sum(-1) @ cur_beta
            cur_g = ng
        g_list.append(cur_g.copy())
        beta_list.append(cur_beta.copy())
    return g_list, beta_list


def _band_matrix(gE, bE, scale, half):
    """[121, 32*LH] band matrix for one (scale, set, half): columns are
    (l_local, c); rows are input time (120) plus the constant-one bias row."""
    Ki = gE.shape[1]
    Lh = LH[scale]
    l0 = 0 if half == 0 else L_LIST[scale] - Lh
    Gm = np.zeros((T + 1, 32 * Lh), dtype=np.float64)
    gT = gE.T  # [Ki, 32]
    for ll in range(Lh):
        l = l0 + ll
        Gm[l:l + Ki, ll * 32:(ll + 1) * 32] = gT
    Gm[T, :] = np.tile(bE, Lh)
    return Gm.astype(np.float32)


def _host_prep(inputs):
    """All host-side preprocessing -> per-core input maps (minus xa)."""
    adjs = _graph_adjs(inputs)
    eye = np.eye(NNODE, dtype=np.float32)
    adjt = np.zeros((8, NPAD, NPAD), dtype=np.float32)
    # scale-0 compaction: the saturated-plateau top-k picks low column
    # indices, so the off-diagonal part of A_0 spans few node chunks.
    used = (adjs[0] > 0).any(0)
    ncol0 = int(np.nonzero(used)[0].max()) + 1 if used.any() else 1
    w0chunks = (ncol0 + 127) // 128
    if w0chunks >= 8:
        w0chunks = 0  # dense fallback: no split for scale 0
    diag0 = np.zeros((2, NPAD, 128), dtype=np.float32)
    for i in range(4):
        for d in range(2):
            m = adjs[i] if d == 0 else adjs[i].T
            a = m + eye
            a = a / a.sum(1, keepdims=True)
            if i == 0 and w0chunks:
                dv = np.diag(a).copy()
                a = a.copy()
                np.fill_diagonal(a, 0.0)   # exact split: A = D + S
                for v in range(NNODE):
                    diag0[d, v, v % 128] = dv[v]
            adjt[i * 2 + d, :NNODE, :NNODE] = a.T
    g_list, beta_list = _composite_convs(inputs)
    al, be = PROPALPHA, 1.0 - PROPALPHA
    gms = {}   # (i, d, j, half) -> [121, F_i] fp32
    for i in range(4):
        for d in range(2):
            wm = np.asarray(inputs["mp1_w" if d == 0 else "mp2_w"][i],
                            dtype=np.float64).reshape(C, 3 * C)
            bm = np.asarray(inputs["mp1_b" if d == 0 else "mp2_b"][i],
                            dtype=np.float64)
            W0, W1, W2 = wm[:, :C], wm[:, C:2 * C], wm[:, 2 * C:]
            E = [W0 + al * (W1 + W2), be * W1 + al * be * W2, (be ** 2) * W2]
            bias = [E[0] @ beta_list[i] + bm, E[1] @ beta_list[i],
                    E[2] @ beta_list[i]]
            for j in range(3):
                gE = E[j] @ g_list[i]
                for half in range(2):
                    gms[(i, d, j, half)] = _band_matrix(gE, bias[j], i, half)
    # scale0: plain conv, kernel length T, bias row
    g00 = np.zeros((T + 1, C), dtype=np.float64)
    g00[:T, :] = np.asarray(inputs["scale0_w"], dtype=np.float64)[:, 0, 0, :].T
    g00[T, :] = np.asarray(inputs["scale0_b"], dtype=np.float64)
    g00 = g00.astype(np.float32)
    return adjt, gms, g00, diag0, w0chunks


def _setup_ntff_hook():
    """Enable NTFF profiling under axon (used only when FPN_TRACE=1)."""
    import types
    if "antenv.axon_hooks" not in sys.modules:
        ah = types.ModuleType("antenv.axon_hooks")
        ah._hook = None
        ah.set_axon_ntff_profile_hook = lambda h: setattr(ah, "_hook", h)
        ah.get_axon_ntff_profile_hook = lambda: ah._hook
        sys.modules["antenv.axon_hooks"] = ah
    ah = sys.modules["antenv.axon_hooks"]
    if getattr(ah, "_hook", None) is None:
        try:
            from trn_agent_boot.trn_boot import _ntff_profile_via_ctypes
            ah.set_axon_ntff_profile_hook(
                _ntff_profile_via_ctypes('/opt/axon/libaxon_pjrt.so'))
        except Exception as e:
            print("ntff hook setup failed:", e, file=sys.stderr)


_NC = {}


def _build_nc(w0chunks):
    if w0chunks in _NC:
        return _NC[w0chunks]
    import concourse.bacc as bacc
    import concourse.mybir as mybir
    from concourse.tile import TileContext

    f32 = mybir.dt.float32
    f32r = mybir.dt.float32r

    nc = bacc.Bacc(None, target_bir_lowering=False)
    xa_d = nc.dram_tensor("xa", (T + 1, NPAD), f32, kind="ExternalInput")
    adjt_d = nc.dram_tensor("adjt", (8, NPAD, NPAD), f32, kind="ExternalInput")
    diag0_d = nc.dram_tensor("diag0", (2, NPAD, 128), f32, kind="ExternalInput")
    g00_d = nc.dram_tensor("g00", (T + 1, C), f32, kind="ExternalInput")
    gm_d = {}
    for i in range(4):
        for d in range(2):
            for j in range(3):
                gm_d[(i, d, j)] = nc.dram_tensor(
                    f"gm_{i}_{d}_{j}", (T + 1, F_LIST[i]), f32,
                    kind="ExternalInput")
    out_d = {}
    for i in range(4):
        for d in range(2):
            out_d[(i, d)] = nc.dram_tensor(
                f"out_{i}_{d}", (NPAD, F_LIST[i]), f32, kind="ExternalOutput")
    out00_d = nc.dram_tensor("out00", (NPAD, C), f32, kind="ExternalOutput")

    FMAX = FCH[0]  # 432

    with TileContext(nc) as tc:
        with tc.tile_pool(name="const", bufs=1) as constp, \
             tc.tile_pool(name="stage", bufs=2) as stage, \
             tc.tile_pool(name="adj", bufs=2) as adjp, \
             tc.tile_pool(name="gp", bufs=2) as gp, \
             tc.tile_pool(name="sp", bufs=2) as sp, \
             tc.tile_pool(name="op", bufs=3) as op, \
             tc.tile_pool(name="ps", bufs=2, space="PSUM") as ps, \
             tc.tile_pool(name="psc3", bufs=3, space="PSUM") as psc3:

            xa_f = constp.tile([T + 1, NPAD], f32, name="xa_f")
            nc.gpsimd.dma_start(xa_f[:], xa_d[:, :])
            xa_r = constp.tile([T + 1, NPAD], f32r, name="xa_r")
            nc.vector.tensor_copy(xa_r[:], xa_f[:])

            # scale0 first: tiny matmuls warm the PE while adjacencies stream
            g00s = stage.tile([T + 1, C], f32, tag="gstage", name="g00s")
            nc.gpsimd.dma_start(g00s[:], g00_d[:, :])
            g00r = constp.tile([T + 1, C], f32r, name="g00r")
            nc.vector.tensor_copy(g00r[:], g00s[:])
            for w in range(8):
                p0 = psc3.tile([128, C], f32, tag="psc", name=f"p0_{w}")
                nc.tensor.matmul(p0[:], xa_r[:, w * 128:(w + 1) * 128],
                                 g00r[:], start=True, stop=True)
                o0 = op.tile([128, C], f32, tag="o0", name=f"o0_{w}")
                nc.vector.tensor_copy(o0[:], p0[:])
                nc.sync.dma_start(out00_d[w * 128:(w + 1) * 128, :], o0[:])

            for i in (1, 2, 3, 0):
                Fi, fc = F_LIST[i], FCH[i]
                compact = (i == 0 and w0chunks > 0)
                for d in range(2):
                    kidx = i * 2 + d
                    gr = []
                    for j in range(3):
                        gs = stage.tile([T + 1, Fi], f32, tag="gstage",
                                        name=f"gst_{i}_{d}_{j}")
                        nc.gpsimd.dma_start(gs[:], gm_d[(i, d, j)][:, :])
                        gj = gp.tile([T + 1, F_LIST[0]], f32r, tag=f"gr{j}",
                                     name=f"gr_{i}_{d}_{j}")[:, :Fi]
                        nc.scalar.copy(gj, gs[:])
                        gr.append(gj)
                    # which stationary w-chunks of S^T are needed, and which
                    # output v-chunks have S contributions
                    if compact and d == 0:
                        wlist = list(range(w0chunks))   # S columns compacted
                        vfull = list(range(8))
                    elif compact and d == 1:
                        wlist = list(range(8))
                        vfull = list(range(w0chunks))   # S rows compacted
                    else:
                        wlist = list(range(8))
                        vfull = list(range(8))
                    adjr = {}
                    for w in wlist:
                        csl = (slice(0, w0chunks * 128) if (compact and d == 1)
                               else slice(0, NPAD))
                        cw = csl.stop - csl.start
                        st = stage.tile([128, NPAD], f32, tag="adjstage",
                                        name=f"ast_{i}_{d}_{w}")[:, :cw]
                        nc.gpsimd.dma_start(
                            st, adjt_d[kidx, w * 128:(w + 1) * 128, csl])
                        ar = adjp.tile([128, NPAD], f32r, tag=f"adjr{w}",
                                       name=f"adjr_{i}_{d}_{w}")[:, :cw]
                        nc.scalar.copy(ar, st)
                        adjr[w] = ar
                    dgr = {}
                    if compact:
                        for v in range(8):
                            dst = stage.tile([128, 128], f32, tag="dstage",
                                             name=f"dst_{d}_{v}")
                            nc.gpsimd.dma_start(
                                dst[:], diag0_d[d, v * 128:(v + 1) * 128, :])
                            dg = adjp.tile([128, 128], f32r, tag=f"dg{v}",
                                           name=f"dg_{d}_{v}")
                            nc.scalar.copy(dg[:], dst[:])
                            dgr[v] = dg

                    def hop(v, g_band, rhs_tiles, psname, f, sl):
                        """One output accumulation group for chunk v."""
                        vs = slice(v * 128, (v + 1) * 128)
                        mm = [("conv", None)]
                        if compact:
                            mm.append(("diag", None))
                            if (d == 0) or (v in vfull):
                                mm += [("adj", w) for w in wlist]
                        else:
                            mm += [("adj", w) for w in wlist]
                        ph = ps.tile([128, FMAX], f32, tag=psname,
                                     name=f"{psname}_{i}_{d}_{f}_{v}")[:, :fc]
                        last = len(mm) - 1
                        for k, (kind, w) in enumerate(mm):
                            if kind == "conv":
                                nc.tensor.matmul(ph, xa_r[:, vs],
                                                 g_band[:, sl],
                                                 start=(k == 0),
                                                 stop=(k == last))
                            elif kind == "diag":
                                nc.tensor.matmul(ph, dgr[v][:],
                                                 rhs_tiles[v],
                                                 start=(k == 0),
                                                 stop=(k == last))
                            else:
                                lh = (adjr[w][:, vs] if not (compact and d == 1)
                                      else adjr[w][:, v * 128:(v + 1) * 128])
                                nc.tensor.matmul(ph, lh, rhs_tiles[w],
                                                 start=(k == 0),
                                                 stop=(k == last))
                        return ph

                    for f in range(4):
                        sl = slice(f * fc, (f + 1) * fc)
                        s2t = []
                        for w in range(8):
                            pc = psc3.tile([128, FMAX], f32, tag="psc",
                                           name=f"pc_{i}_{d}_{f}_{w}")[:, :fc]
                            nc.tensor.matmul(
                                pc, xa_r[:, w * 128:(w + 1) * 128],
                                gr[2][:, sl], start=True, stop=True)
                            s2 = sp.tile([128, FMAX], f32r, tag=f"s2_{w}",
                                         name=f"s2_{i}_{d}_{f}_{w}")[:, :fc]
                            nc.vector.tensor_copy(s2, pc)
                            s2t.append(s2)
                        ur = []
                        for v in range(8):
                            ph = hop(v, gr[1], s2t, "ps1", f, sl)
                            u = sp.tile([128, FMAX], f32r, tag=f"u_{v}",
                                        name=f"u_{i}_{d}_{f}_{v}")[:, :fc]
                            nc.vector.tensor_copy(u, ph)
                            ur.append(u)
                        for v in range(8):
                            ph2 = hop(v, gr[0], ur, "ps2", f, sl)
                            ob = op.tile([128, FMAX], f32, tag="ob",
                                         name=f"ob_{i}_{d}_{f}_{v}")[:, :fc]
                            nc.vector.tensor_copy(ob, ph2)
                            nc.sync.dma_start(
                                out_d[(i, d)][v * 128:(v + 1) * 128, sl], ob)
    nc.compile()
    _NC[w0chunks] = nc
    return nc


def kernel(**inputs):
    inputs = {k: np.asarray(v) for k, v in inputs.items()}
    adjt, gms, g00, diag0, w0chunks = _host_prep(inputs)
    x = inputs["x"].astype(np.float32)  # [B, T, N]

    nc = _build_nc(w0chunks)
    in_maps = []
    for b in range(B):
        for half in range(2):
            xa = np.zeros((T + 1, NPAD), dtype=np.float32)
            xa[:T, :NNODE] = x[b]
            xa[T, :NNODE] = 1.0
            m = {"xa": xa, "adjt": adjt, "g00": g00, "diag0": diag0}
            for i in range(4):
                for d in range(2):
                    for j in range(3):
                        m[f"gm_{i}_{d}_{j}"] = gms[(i, d, j, half)]
            in_maps.append(m)

    from concourse.bass_utils import run_bass_kernel_spmd
    trace = bool(os.environ.get("FPN_TRACE"))
    if trace:
        _setup_ntff_hook()
    r = run_bass_kernel_spmd(nc, in_maps, core_ids=list(range(8)),
                             trace=trace)
    kernel.last_exec_ns = r.exec_time_ns
    kernel.last_insts = r.instructions_and_trace
    res = r.results

    outs = []
    s0 = np.empty((B, C, NNODE, 1), dtype=np.float32)
    for b in range(B):
        s0[b, :, :, 0] = res[b * 2]["out00"][:NNODE, :].T
    outs.append(s0)
    for i in range(4):
        Li, Lhi = L_LIST[i], LH[i]
        full = np.empty((B, C, NNODE, Li), dtype=np.float32)
        for b in range(B):
            for half in range(2):
                r = res[b * 2 + half]
                part = r[f"out_{i}_0"][:NNODE] + r[f"out_{i}_1"][:NNODE]
                # [N, Lh*32] -> [C, N, Lh]
                part = part.reshape(NNODE, Lhi, C).transpose(2, 0, 1)
                if half == 0:
                    full[b, :, :, :Lhi] = part
                else:
                    j0 = 2 * Lhi - Li
                    full[b, :, :, Lhi:] = part[:, :, j0:]
        outs.append(full)
    return tuple(outs)


# revision 16
# speedup vs baseline: 1.0625x; 1.0107x over previous
"""FPN encoder (MTGNN/MAGNN-style) Trainium2 kernel.

Strategy:
 - Host: graph constructor (tiny, input-only-dependent, exact-tie-sensitive
   top-k) computed bit-exactly with jax-CPU in a subprocess; adjacencies are
   then row-normalized, transposed, padded to 1024 and replicated to all
   cores like weights. The linear FPN conv pyramid and the mixprop 1x1-conv
   channel mixes are folded (host-side weight preprocessing) into per-scale
   band matrices G0/G1/G2 of one composite temporal conv each, with biases as
   an extra constant-one row of x.
 - Device (8 cores, zero collectives): core (b, h) handles batch b and the
   h-th half of the time axis of every scale. Everything is node-major
   matmuls on the tensor engine in float32r:
       S2 = xa^T G2 ; u = A (S2) + xa^T G1 ; out_d = A u + xa^T G0
   with A-chunks as stationary operands, two hops per scale per direction.
 - Host gather: slice away node padding, fold the two directions' partial
   sums, reassemble the 5-tuple of full outputs.
"""

import os
import sys
import subprocess
import tempfile

import numpy as np

for _p in ('/opt/trn_rl_repo', '/root/.axon_site/_ro/trn_rl_repo'):
    if os.path.isdir(_p) and _p not in sys.path:
        sys.path.append(_p)

B, T, NNODE, C, D = 4, 120, 1000, 32, 40
KERNEL_SET = [14, 7, 6, 3]
PROPALPHA, ALPHA, TOPK = 0.05, 3.0, 20
SCALE_SET = [1.0, 0.8, 0.6, 0.5]
NPAD = 1024
L_LIST = [107, 101, 96, 94]
LH = [54, 51, 48, 47]          # per-scale half lengths (ceil(L/2))
F_LIST = [32 * l for l in LH]  # free width per scale: 1728,1632,1536,1504
FCH = [f // 4 for f in F_LIST]  # free chunk: 432,408,384,376 (256..512)
GLEN = [14, 20, 25, 27]        # composite conv kernel lengths

_CHILD = r'''
import sys, numpy as np
import jax, jax.numpy as jnp
pin, pout = sys.argv[1], sys.argv[2]
z = np.load(pin)
emb1, emb2 = jnp.asarray(z["emb1"]), jnp.asarray(z["emb2"])
l1w, l1b = jnp.asarray(z["lin1_w"]), jnp.asarray(z["lin1_b"])
l2w, l2b = jnp.asarray(z["lin2_w"]), jnp.asarray(z["lin2_b"])
NN, KK, AL = 1000, 20, 3.0
SC = [1.0, 0.8, 0.6, 0.5]
n1, n2 = emb1, emb2
rows = jnp.arange(NN)[:, None]
adjs = []
for i in range(4):
    n1 = jnp.tanh(AL * ((n1 * SC[i]) @ l1w[i].T + l1b[i]))
    n2 = jnp.tanh(AL * ((n2 * SC[i]) @ l2w[i].T + l2b[i]))
    a = n1 @ n2.T - n2 @ n1.T
    adj0 = jax.nn.relu(jnp.tanh(AL * a))
    _, t1 = jax.lax.top_k(adj0, KK)
    mask = jnp.zeros_like(adj0).at[rows, t1].set(1.0)
    adjs.append(np.asarray(adj0 * mask))
np.savez(pout, **{f"a{i}": adjs[i] for i in range(4)})
'''


def _graph_adjs(inputs):
    """Replicate reference.graph_construct bit-exactly on jax-CPU."""
    with tempfile.TemporaryDirectory() as td:
        pin = os.path.join(td, "in.npz")
        pout = os.path.join(td, "out.npz")
        np.savez(pin, emb1=inputs["emb1"], emb2=inputs["emb2"],
                 lin1_w=inputs["lin1_w"], lin1_b=inputs["lin1_b"],
                 lin2_w=inputs["lin2_w"], lin2_b=inputs["lin2_b"])
        env = dict(os.environ)
        env["JAX_PLATFORMS"] = "cpu"
        env.pop("TRN_TERMINAL_POOL_IPS", None)
        parts = []
        for chunk in (env.get("PYTHONPATH", ""), env.get("NIX_PYTHONPATH", "")):
            parts.extend(p for p in chunk.split(os.pathsep) if p)
        parts.extend(p for p in sys.path if p)
        env["PYTHONPATH"] = os.pathsep.join(dict.fromkeys(parts))
        r = subprocess.run([sys.executable, "-c", _CHILD, pin, pout],
                           env=env, capture_output=True, text=True, timeout=900)
        if r.returncode != 0:
            raise RuntimeError(f"graph subprocess failed:\n{r.stdout}\n{r.stderr}")
        z = np.load(pout)
        return [z[f"a{i}"] for i in range(4)]


def _composite_convs(inputs):
    """Compose the linear FPN pyramid into one temporal kernel per scale.

    Returns g_list[i] [32, GLEN[i]] and beta_list[i] [32] (fp64) such that
    scales[i][c, n, l] = sum_k g[c, k] x[l + k, n] + beta[c].
    """
    g_list, beta_list = [], []
    cur_g = None
    cur_beta = None
    for idx, k in enumerate(KERNEL_SET):
        w = np.asarray(inputs[f"msc_w{idx}"], dtype=np.float64)
        b = np.asarray(inputs[f"msc_b{idx}"], dtype=np.float64)
        if idx == 0:
            cur_g = w[:, 0, 0, :].copy()
            cur_beta = b.copy()
        else:
            kp = cur_g.shape[1]
            ng = np.zeros((C, kp + k - 1))
            for t2 in range(k):
                ng[:, t2:t2 + kp] += w[:, :, 0, t2] @ cur_g
            cur_beta = b + w[:, :, 0, :].sum(-1) @ cur_beta
            cur_g = ng
        g_list.append(cur_g.copy())
        beta_list.append(cur_beta.copy())
    return g_list, beta_list


def _band_matrix(gE, bE, scale, half):
    """[121, 32*LH] band matrix for one (scale, set, half): columns are
    (l_local, c); rows are input time (120) plus the constant-one bias row."""
    Ki = gE.shape[1]
    Lh = LH[scale]
    l0 = 0 if half == 0 else L_LIST[scale] - Lh
    Gm = np.zeros((T + 1, 32 * Lh), dtype=np.float64)
    gT = gE.T  # [Ki, 32]
    for ll in range(Lh):
        l = l0 + ll
        Gm[l:l + Ki, ll * 32:(ll + 1) * 32] = gT
    Gm[T, :] = np.tile(bE, Lh)
    return Gm.astype(np.float32)


def _host_prep(inputs):
    """All host-side preprocessing -> per-core input maps (minus xa)."""
    adjs = _graph_adjs(inputs)
    eye = np.eye(NNODE, dtype=np.float32)
    adjt = np.zeros((8, NPAD, NPAD), dtype=np.float32)
    # scale-0 compaction: the saturated-plateau top-k picks low column
    # indices, so the off-diagonal part of A_0 spans few node chunks.
    used = (adjs[0] > 0).any(0)
    ncol0 = int(np.nonzero(used)[0].max()) + 1 if used.any() else 1
    w0chunks = (ncol0 + 127) // 128
    if w0chunks >= 8:
        w0chunks = 0  # dense fallback: no split for scale 0
    diag0 = np.zeros((2, NPAD, 128), dtype=np.float32)
    for i in range(4):
        for d in range(2):
            m = adjs[i] if d == 0 else adjs[i].T
            a = m + eye
            a = a / a.sum(1, keepdims=True)
            if i == 0 and w0chunks:
                dv = np.diag(a).copy()
                a = a.copy()
                np.fill_diagonal(a, 0.0)   # exact split: A = D + S
                for v in range(NNODE):
                    diag0[d, v, v % 128] = dv[v]
            adjt[i * 2 + d, :NNODE, :NNODE] = a.T
    g_list, beta_list = _composite_convs(inputs)
    al, be = PROPALPHA, 1.0 - PROPALPHA
    gms = {}   # (i, d, j, half) -> [121, F_i] fp32
    for i in range(4):
        for d in range(2):
            wm = np.asarray(inputs["mp1_w" if d == 0 else "mp2_w"][i],
                            dtype=np.float64).reshape(C, 3 * C)
            bm = np.asarray(inputs["mp1_b" if d == 0 else "mp2_b"][i],
                            dtype=np.float64)
            W0, W1, W2 = wm[:, :C], wm[:, C:2 * C], wm[:, 2 * C:]
            E = [W0 + al * (W1 + W2), be * W1 + al * be * W2, (be ** 2) * W2]
            bias = [E[0] @ beta_list[i] + bm, E[1] @ beta_list[i],
                    E[2] @ beta_list[i]]
            for j in range(3):
                gE = E[j] @ g_list[i]
                for half in range(2):
                    gms[(i, d, j, half)] = _band_matrix(gE, bias[j], i, half)
    # scale0: plain conv, kernel length T, bias row
    g00 = np.zeros((T + 1, C), dtype=np.float64)
    g00[:T, :] = np.asarray(inputs["scale0_w"], dtype=np.float64)[:, 0, 0, :].T
    g00[T, :] = np.asarray(inputs["scale0_b"], dtype=np.float64)
    g00 = g00.astype(np.float32)
    return adjt, gms, g00, diag0, w0chunks


def _setup_ntff_hook():
    """Enable NTFF profiling under axon (used only when FPN_TRACE=1)."""
    import types
    if "antenv.axon_hooks" not in sys.modules:
        ah = types.ModuleType("antenv.axon_hooks")
        ah._hook = None
        ah.set_axon_ntff_profile_hook = lambda h: setattr(ah, "_hook", h)
        ah.get_axon_ntff_profile_hook = lambda: ah._hook
        sys.modules["antenv.axon_hooks"] = ah
    ah = sys.modules["antenv.axon_hooks"]
    if getattr(ah, "_hook", None) is None:
        try:
            from trn_agent_boot.trn_boot import _ntff_profile_via_ctypes
            ah.set_axon_ntff_profile_hook(
                _ntff_profile_via_ctypes('/opt/axon/libaxon_pjrt.so'))
        except Exception as e:
            print("ntff hook setup failed:", e, file=sys.stderr)


_NC = {}


def _build_nc(w0chunks):
    if w0chunks in _NC:
        return _NC[w0chunks]
    import concourse.bacc as bacc
    import concourse.mybir as mybir
    from concourse.tile import TileContext

    f32 = mybir.dt.float32
    f32r = mybir.dt.float32r

    nc = bacc.Bacc(None, target_bir_lowering=False)
    xa_d = nc.dram_tensor("xa", (T + 1, NPAD), f32, kind="ExternalInput")
    adjt_d = nc.dram_tensor("adjt", (8, NPAD, NPAD), f32, kind="ExternalInput")
    diag0_d = nc.dram_tensor("diag0", (2, NPAD, 128), f32, kind="ExternalInput")
    g00_d = nc.dram_tensor("g00", (T + 1, C), f32, kind="ExternalInput")
    gm_d = {}
    for i in range(4):
        for d in range(2):
            for j in range(3):
                gm_d[(i, d, j)] = nc.dram_tensor(
                    f"gm_{i}_{d}_{j}", (T + 1, F_LIST[i]), f32,
                    kind="ExternalInput")
    out_d = {}
    for i in range(4):
        for d in range(2):
            out_d[(i, d)] = nc.dram_tensor(
                f"out_{i}_{d}", (NPAD, F_LIST[i]), f32, kind="ExternalOutput")
    out00_d = nc.dram_tensor("out00", (NPAD, C), f32, kind="ExternalOutput")

    FMAX = FCH[0]  # 432

    with TileContext(nc) as tc:
        with tc.tile_pool(name="const", bufs=1) as constp, \
             tc.tile_pool(name="stage", bufs=2) as stage, \
             tc.tile_pool(name="adj", bufs=2) as adjp, \
             tc.tile_pool(name="gp", bufs=2) as gp, \
             tc.tile_pool(name="sp", bufs=2) as sp, \
             tc.tile_pool(name="op", bufs=3) as op, \
             tc.tile_pool(name="ps", bufs=2, space="PSUM") as ps, \
             tc.tile_pool(name="psc3", bufs=3, space="PSUM") as psc3:

            xa_f = constp.tile([T + 1, NPAD], f32, name="xa_f")
            nc.gpsimd.dma_start(xa_f[:], xa_d[:, :])
            xa_r = constp.tile([T + 1, NPAD], f32r, name="xa_r")
            nc.vector.tensor_copy(xa_r[:], xa_f[:])

            # scale0 first: tiny matmuls warm the PE while adjacencies stream
            g00s = stage.tile([T + 1, C], f32, tag="gstage", name="g00s")
            nc.gpsimd.dma_start(g00s[:], g00_d[:, :])
            g00r = constp.tile([T + 1, C], f32r, name="g00r")
            nc.vector.tensor_copy(g00r[:], g00s[:])
            for w in range(8):
                p0 = psc3.tile([128, C], f32, tag="psc", name=f"p0_{w}")
                nc.tensor.matmul(p0[:], xa_r[:, w * 128:(w + 1) * 128],
                                 g00r[:], start=True, stop=True)
                o0 = op.tile([128, C], f32, tag="o0", name=f"o0_{w}")
                nc.vector.tensor_copy(o0[:], p0[:])
                nc.sync.dma_start(out00_d[w * 128:(w + 1) * 128, :], o0[:])

            for i in (1, 2, 3, 0):
                Fi, fc = F_LIST[i], FCH[i]
                compact = (i == 0 and w0chunks > 0)
                for d in range(2):
                    kidx = i * 2 + d
                    gr = [None, None, None]
                    for j in (2, 1, 0):   # G2 first: S2 convs only need it
                        gs = stage.tile([T + 1, Fi], f32, tag="gstage",
                                        name=f"gst_{i}_{d}_{j}")
                        nc.gpsimd.dma_start(gs[:], gm_d[(i, d, j)][:, :])
                        gj = gp.tile([T + 1, F_LIST[0]], f32r, tag=f"gr{j}",
                                     name=f"gr_{i}_{d}_{j}")[:, :Fi]
                        nc.scalar.copy(gj, gs[:])
                        gr[j] = gj
                    # which stationary w-chunks of S^T are needed, and which
                    # output v-chunks have S contributions
                    if compact and d == 0:
                        wlist = list(range(w0chunks))   # S columns compacted
                        vfull = list(range(8))
                    elif compact and d == 1:
                        wlist = list(range(8))
                        vfull = list(range(w0chunks))   # S rows compacted
                    else:
                        wlist = list(range(8))
                        vfull = list(range(8))
                    adjr = {}
                    for w in wlist:
                        csl = (slice(0, w0chunks * 128) if (compact and d == 1)
                               else slice(0, NPAD))
                        cw = csl.stop - csl.start
                        st = stage.tile([128, NPAD], f32, tag="adjstage",
                                        name=f"ast_{i}_{d}_{w}")[:, :cw]
                        nc.gpsimd.dma_start(
                            st, adjt_d[kidx, w * 128:(w + 1) * 128, csl])
                        ar = adjp.tile([128, NPAD], f32r, tag=f"adjr{w}",
                                       name=f"adjr_{i}_{d}_{w}")[:, :cw]
                        nc.scalar.copy(ar, st)
                        adjr[w] = ar
                    dgr = {}
                    if compact:
                        for v in range(8):
                            dst = stage.tile([128, 128], f32, tag="dstage",
                                             name=f"dst_{d}_{v}")
                            nc.gpsimd.dma_start(
                                dst[:], diag0_d[d, v * 128:(v + 1) * 128, :])
                            dg = adjp.tile([128, 128], f32r, tag=f"dg{v}",
                                           name=f"dg_{d}_{v}")
                            nc.scalar.copy(dg[:], dst[:])
                            dgr[v] = dg

                    def hop(v, g_band, rhs_tiles, psname, f, sl):
                        """One output accumulation group for chunk v."""
                        vs = slice(v * 128, (v + 1) * 128)
                        mm = [("conv", None)]
                        if compact:
                            mm.append(("diag", None))
                            if (d == 0) or (v in vfull):
                                mm += [("adj", w) for w in wlist]
                        else:
                            mm += [("adj", w) for w in wlist]
                        ph = ps.tile([128, FMAX], f32, tag=psname,
                                     name=f"{psname}_{i}_{d}_{f}_{v}")[:, :fc]
                        last = len(mm) - 1
                        for k, (kind, w) in enumerate(mm):
                            if kind == "conv":
                                nc.tensor.matmul(ph, xa_r[:, vs],
                                                 g_band[:, sl],
                                                 start=(k == 0),
                                                 stop=(k == last))
                            elif kind == "diag":
                                nc.tensor.matmul(ph, dgr[v][:],
                                                 rhs_tiles[v],
                                                 start=(k == 0),
                                                 stop=(k == last))
                            else:
                                lh = (adjr[w][:, vs] if not (compact and d == 1)
                                      else adjr[w][:, v * 128:(v + 1) * 128])
                                nc.tensor.matmul(ph, lh, rhs_tiles[w],
                                                 start=(k == 0),
                                                 stop=(k == last))
                        return ph

                    for f in range(4):
                        sl = slice(f * fc, (f + 1) * fc)
                        s2t = []
                        for w in range(8):
                            pc = psc3.tile([128, FMAX], f32, tag="psc",
                                           name=f"pc_{i}_{d}_{f}_{w}")[:, :fc]
                            nc.tensor.matmul(
                                pc, xa_r[:, w * 128:(w + 1) * 128],
                                gr[2][:, sl], start=True, stop=True)
                            s2 = sp.tile([128, FMAX], f32r, tag=f"s2_{w}",
                                         name=f"s2_{i}_{d}_{f}_{w}")[:, :fc]
                            nc.vector.tensor_copy(s2, pc)
                            s2t.append(s2)
                        ur = []
                        for v in range(8):
                            ph = hop(v, gr[1], s2t, "ps1", f, sl)
                            u = sp.tile([128, FMAX], f32r, tag=f"u_{v}",
                                        name=f"u_{i}_{d}_{f}_{v}")[:, :fc]
                            nc.vector.tensor_copy(u, ph)
                            ur.append(u)
                        for v in range(8):
                            ph2 = hop(v, gr[0], ur, "ps2", f, sl)
                            ob = op.tile([128, FMAX], f32, tag="ob",
                                         name=f"ob_{i}_{d}_{f}_{v}")[:, :fc]
                            nc.vector.tensor_copy(ob, ph2)
                            nc.sync.dma_start(
                                out_d[(i, d)][v * 128:(v + 1) * 128, sl], ob)
    nc.compile()
    _NC[w0chunks] = nc
    return nc


def kernel(**inputs):
    inputs = {k: np.asarray(v) for k, v in inputs.items()}
    adjt, gms, g00, diag0, w0chunks = _host_prep(inputs)
    x = inputs["x"].astype(np.float32)  # [B, T, N]

    nc = _build_nc(w0chunks)
    in_maps = []
    for b in range(B):
        for half in range(2):
            xa = np.zeros((T + 1, NPAD), dtype=np.float32)
            xa[:T, :NNODE] = x[b]
            xa[T, :NNODE] = 1.0
            m = {"xa": xa, "adjt": adjt, "g00": g00, "diag0": diag0}
            for i in range(4):
                for d in range(2):
                    for j in range(3):
                        m[f"gm_{i}_{d}_{j}"] = gms[(i, d, j, half)]
            in_maps.append(m)

    from concourse.bass_utils import run_bass_kernel_spmd
    trace = bool(os.environ.get("FPN_TRACE"))
    if trace:
        _setup_ntff_hook()
    r = run_bass_kernel_spmd(nc, in_maps, core_ids=list(range(8)),
                             trace=trace)
    kernel.last_exec_ns = r.exec_time_ns
    kernel.last_insts = r.instructions_and_trace
    res = r.results

    outs = []
    s0 = np.empty((B, C, NNODE, 1), dtype=np.float32)
    for b in range(B):
        s0[b, :, :, 0] = res[b * 2]["out00"][:NNODE, :].T
    outs.append(s0)
    for i in range(4):
        Li, Lhi = L_LIST[i], LH[i]
        full = np.empty((B, C, NNODE, Li), dtype=np.float32)
        for b in range(B):
            for half in range(2):
                r = res[b * 2 + half]
                part = r[f"out_{i}_0"][:NNODE] + r[f"out_{i}_1"][:NNODE]
                # [N, Lh*32] -> [C, N, Lh]
                part = part.reshape(NNODE, Lhi, C).transpose(2, 0, 1)
                if half == 0:
                    full[b, :, :, :Lhi] = part
                else:
                    j0 = 2 * Lhi - Li
                    full[b, :, :, Lhi:] = part[:, :, j0:]
        outs.append(full)
    return tuple(outs)
